# revision 40
# baseline (speedup 1.0000x reference)
"""DeformableAttention1D on 8 TRN2 NeuronCores via Bass/Tile.

Sharding: core c handles offset-group g=c//2 (64 of 256 channels, 2 of 8 heads)
and query-half qh=c%2 (512 of 1024 positions). Each core computes its group's
offsets/gather/bias/attention independently; the final output projection is
computed as a partial (wo sliced by group) and summed on the host (the
"all-reduce" of the output projection).

Key idea vs the straightforward implementation: the CPB relative-position-bias
MLP is a scalar->2 function F(d) of the signed distance d = gq_i - vgsp1_j,
and the query grid steps uniformly by h_q = 2/1023. Tabulating F on a grid
with spacing exactly h_q (a host-side weights-only precompute, like any other
weight repacking) turns the bias into

    bias[i, j] = (1-r_j) T[m_j + i] + r_j T[m_j + 1 + i],   m_j + r_j = c_j,

i.e. a per-column shifted window of the table. On device this is 9 matmuls of
"tent" interpolation one-hots max(0, 1-|c_j - kappa|) (stationary) against
host-precomputed Hankel slabs of T (moving), accumulated directly into the
attention-logit PSUM on top of q.k — the bias costs zero vector-engine work
in the attention phase. The tent matrices also implement the bilinear kv
grid-sample gather (zeros padding included). Tents are only materialized in
their static nonzero j-windows (~43 of 256 columns per kappa-tile).

Device numerics: fp32 data, fp32r matmuls (1 cycle/col vs 4 for fp32). The
ACT engine is restricted to ONE table set (natural_log_exp_and_others:
Exp/Ln/Relu/Copy/Identity/Square); tanh and erf(gelu) are composed from
Exp + DVE ops. Max bias interp error ~3e-4 in the logits.
"""
import os
import sys

sys.path.insert(0, "/opt/trn_rl_repo")

import numpy as np

import concourse.bacc as bacc
import concourse.bass as bass
import concourse.mybir as mybir
import concourse.tile as tile
import concourse.bass_utils as bass_utils

F32 = mybir.dt.float32
F32R = mybir.dt.float32r
I32 = mybir.dt.int32
U32 = mybir.dt.uint32
AF = mybir.ActivationFunctionType
ALU = mybir.AluOpType

# model dims (hardcoded per problem spec)
DIM = 256
N = 1024
G = 4
HEADS = 8
DH = 32
NDS = 256          # downsampled kv positions
QS = 512           # queries per core
DPG = 64           # channels per group
OFF_K = 6
DS = 4             # downsample stride
OFF_SCALE = 4.0
NCORES = 8

# bias lookup table
HQ = 2.0 / 1023.0  # query grid step == table spacing
CQ = 1040.0        # index offset so c_j = CQ - vgsp1_j/HQ stays in [0.9, 1057]
LTAB = 1664        # table length (slabs need up to 128*8+127+511 = 1662)
NTT = 9            # tent kappa-tiles (c_j+1 < 1152)
NT = 8             # x position tiles for the kv gather
A_S = 1024.0 / 255.0   # d ppix / d vg_raw  (|offset| < 4 -> +-4*A_S slack)
A_O = 1023.0 / 255.0   # -d c / d vg_raw

# A&S 7.1.25 3-term erf coefficients (|err| <= 2.5e-5)
ERF_P3 = 0.47047
ERF_A3 = [0.3480242, -0.0958798, 0.7478556]

# PE warmup chain lengths (tuned against the cost-model timeline)
WARM_A = 3
WARM_B = 6
WARM_C = 6

_CACHED = {}


def _s_window(t):
    """Static j-range where the kv-gather tent for position tile t can be
    nonzero: ppix_j in (128t-1, 128t+128), ppix = A_S*vg_raw-0.5, vg_raw in
    (j-4, j+4)."""
    jlo = max(0, int(np.floor((128 * t - 0.5) / A_S - 4)) - 1)
    jhi = min(NDS, int(np.ceil((128 * t + 128.5) / A_S + 4)) + 1)
    return jlo, jhi


def _o_window(t):
    """Static j-range where the bias tent for kappa tile t can be nonzero:
    c_j in (128t-1, 128t+128), c = CQ - A_O*vg_raw."""
    jlo = max(0, int(np.floor((912 - 128 * t) / A_O - 4)) - 1)
    jhi = min(NDS, int(np.ceil((1041 - 128 * t) / A_O + 4)) + 1)
    return jlo, jhi


def _patch_act_tables():
    """Restrict activation-table selection to the single set that covers all
    ACT functions used by this kernel, so exactly one table load is emitted
    (runtime table swaps do not work in this environment)."""
    import concourse.hw_specs as hw_specs

    if getattr(bacc, "_deform_act_patch", False):
        return
    orig = hw_specs.get_activation_tables

    keep = "natural_log_exp_and_others"

    def patched(module_arch):
        tabs = orig(module_arch)
        keep_funcs = tabs[keep]
        out = {}
        for name, funcs in tabs.items():
            if name == keep:
                out[name] = funcs
            else:
                out[name] = funcs - keep_funcs
        return out

    bacc.get_activation_tables = patched
    bacc._deform_act_patch = True


def _erf_gelu(nc, sb, out_ap, x_ap, shape):
    """Returns (xu, r) with xu + r = x * (1 + erf(x/sqrt(2))).

    A&S 7.1.25 3-term for erf(|x|/sqrt2) = 1 - poly(t)*exp(-x^2/2),
    t = 1/(1 + p*|x|/sqrt2). Sign handled without bit tricks:
    with u = poly*e in (0, 1]:  x*(1+erf(x)) = x*u + relu(2*x*(1-u)).
    """
    P, Nf = shape
    sq = sb.tile([P, Nf], F32, name="gelu_sq", tag="gelu_sq")
    nc.scalar.activation(sq[:], x_ap, AF.Square)
    e = sb.tile([P, Nf], F32, name="gelu_e", tag="gelu_e")
    nc.scalar.activation(e[:], sq[:], AF.Exp, scale=-0.5)
    ax = sb.tile([P, Nf], F32, name="gelu_ax", tag="gelu_ax")
    nc.vector.scalar_tensor_tensor(ax[:], x_ap, -1.0, x_ap, ALU.mult, ALU.max)
    t = sb.tile([P, Nf], F32, name="gelu_t", tag="gelu_t")
    nc.vector.tensor_scalar(t[:], ax[:], float(ERF_P3 / np.sqrt(2.0)), 1.0, ALU.mult, ALU.add)
    nc.vector.reciprocal(t[:], t[:])
    poly = sb.tile([P, Nf], F32, name="gelu_poly", tag="gelu_poly")
    # poly = ((a3 t + a2) t + a1) t
    nc.vector.tensor_scalar(poly[:], t[:], ERF_A3[2], ERF_A3[1], ALU.mult, ALU.add)
    nc.vector.tensor_tensor(poly[:], poly[:], t[:], ALU.mult)
    nc.vector.scalar_tensor_tensor(poly[:], poly[:], ERF_A3[0], t[:], ALU.add, ALU.mult)
    u = sb.tile([P, Nf], F32, name="gelu_u", tag="gelu_u")
    nc.vector.tensor_tensor(u[:], poly[:], e[:], ALU.mult)
    xu = sb.tile([P, Nf], F32R, name="gelu_xu", tag="gelu_xu")
    nc.vector.tensor_tensor(xu[:], x_ap, u[:], ALU.mult)
    w1 = sb.tile([P, Nf], F32, name="gelu_w1", tag="gelu_w1")
    nc.vector.tensor_tensor(w1[:], x_ap, xu[:].bitcast(F32), ALU.subtract)
    r = sb.tile([P, Nf], F32R, name="gelu_r", tag="gelu_r")
    nc.scalar.activation(r[:], w1[:], AF.Relu, scale=2.0)
    return xu, r


def _vg_from_proj(nc, sb, out_ap, x_ap, iotaj4_ap, shape):
    """out = j + 4*tanh(x), tanh = 1 - 2/(exp(2*clamp(x,-15,15))+1):
    out = (iotaj + 4) - 8/(exp(2*xc)+1)."""
    P, Nf = shape
    xc = sb.tile([P, Nf], F32, name="th_xc", tag="th_xc")
    nc.vector.tensor_scalar(xc[:], x_ap, -15.0, 15.0, ALU.max, ALU.min)
    e = sb.tile([P, Nf], F32, name="th_e", tag="th_e")
    nc.scalar.activation(e[:], xc[:], AF.Exp, scale=2.0)
    nc.vector.tensor_scalar(e[:], e[:], 1.0, None, ALU.add)
    r = sb.tile([P, Nf], F32, name="th_r", tag="th_r")
    nc.vector.reciprocal(r[:], e[:])
    nc.vector.scalar_tensor_tensor(out_ap, r[:], -8.0, iotaj4_ap, ALU.mult, ALU.add)


# packed0 (f32r, [128, P0K]): early stationaries. The offsets conv is folded
# into the q projection: acc[ch, m] = sum_{i,kk} wq[ch,i] wdw[ch,kk] x[i,4m+kk-1]
# = sum_c Wc_c^T Xc where Wc_c[(kk,i), ch] and Xc = stride-4 views of xg3.
PK_WQTS = 0         # wq[g].T * DH^-0.5     [64, 64]
PK_WC = 64          # conv stationaries     [128, 3*64]
P0K = 256
# packedr (f32r, [128, PRK]): remaining stationaries (offsets rel. PK_QEND)
PK_QEND = 320
PK_WKT = 320        # wk[g].T               [64, 64]
PK_WVT = 384        # wv[g].T               [64, 64]
PK_WOT = 448        # wo[:, group cols].T   [64, 256]
PK_WEND = 704
PK_WPJ = 704        # 0.5*w_off_proj        [64, 1]
PRK = PK_WEND - PK_QEND + 1
# packed (f32, [128, PCK]): non-matmul constants
PK_BDW = 0          # conv bias             [64, 1]
PK_EYE = 1          # identity              [64, 64]
PCK = 65


def build_nc():
    _patch_act_tables()
    nc = bacc.Bacc("TRN2", target_bir_lowering=False, debug=False, num_devices=NCORES)

    # ---- per-core DRAM inputs ----
    din = {}

    def dt_in(name, shape, dt=F32):
        din[name] = nc.dram_tensor(name, shape, dt, kind="ExternalInput")
        return din[name]

    dt_in("packed0", [128, P0K], F32R)
    dt_in("packedr", [128, PRK], F32R)
    dt_in("xg3", [128, N + 1], F32R)
    dt_in("xq", [DPG, QS], F32R)
    dt_in("xgT", [128, NT * DPG], F32R)
    dt_in("packed", [128, PCK])
    dt_in("hank", [128, 2 * 1536], F32R)
    y_out = nc.dram_tensor("y", [DIM, QS], F32, kind="ExternalOutput")

    with tile.TileContext(nc) as tc:
        with (
            tc.tile_pool(name="const", bufs=1) as cst,
            tc.tile_pool(name="work", bufs=2) as wk,
            tc.tile_pool(name="rows", bufs=1) as rw,
            tc.tile_pool(name="persist", bufs=1) as pe_pool,
        ):
            # ---- input DMAs; slab split across both HWDGE queues ----
            packed0 = cst.tile([128, P0K], F32R, name="packed0", tag="packed0")
            nc.sync.dma_start(packed0[:], din["packed0"].ap())
            # xg3: x duplicated across partition halves with the kk-pair shift
            # baked in: xg3[p, j] = x[p%64, j + p//64 - 1] (zeros off the ends)
            xg3 = cst.tile([128, N + 1], F32R, name="xg3", tag="xg3")
            nc.sync.dma_start(xg3[:], din["xg3"].ap())
            packed = cst.tile([128, PCK], F32, name="packed", tag="packed")
            nc.sync.dma_start(packed[:], din["packed"].ap())
            packedr = cst.tile([128, PRK], F32R, name="packedr", tag="packedr")
            nc.sync.dma_start(packedr[:], din["packedr"].ap())
            xq = cst.tile([DPG, QS], F32R, name="xq", tag="xq")
            nc.sync.dma_start(xq[:], din["xq"].ap())
            xgT = cst.tile([128, NT * DPG], F32R, name="xgT", tag="xgT")
            nc.sync.dma_start(xgT[:], din["xgT"].ap())
            # Hankel strip HH[k', c] = T[k' + c]; the 9 Hankel "slabs" per head
            # are its sliding 512-col windows, so one DMA covers them all
            hank = cst.tile([128, 2 * 1536], F32R, name="hank", tag="hank")
            nc.sync.dma_start(hank[:], din["hank"].ap())



            def W(col, width):
                return packedr[0:DPG, col - PK_QEND:col - PK_QEND + width]

            eye64 = packed[0:DPG, PK_EYE:PK_EYE + DPG]

            # small Pool ops + iotas first (the big tent memsets would
            # head-of-line-block the iotas and, through them, the DVE queue)
            ones_col = cst.tile([128, 1], F32, name="ones", tag="ones")
            nc.gpsimd.memset(ones_col[:], 1.0)
            ones32 = cst.tile([1, 32], F32, name="ones32", tag="ones32")
            nc.gpsimd.memset(ones32[:], 1.0)
            wz = cst.tile([128, 256], F32, name="wz", tag="wz")
            nc.gpsimd.memset(wz[:], 0.0)
            iotaS = cst.tile([128, NT], I32, name="iotaS", tag="iotaS")
            nc.gpsimd.iota(iotaS[:], pattern=[[128, NT]], base=0, channel_multiplier=1)
            iotaO = cst.tile([128, NTT], I32, name="iotaO", tag="iotaO")
            nc.gpsimd.iota(iotaO[:], pattern=[[128, NTT]], base=0, channel_multiplier=1)
            iotaj = rw.tile([1, NDS], I32, name="iotaj", tag="iotaj")
            nc.gpsimd.iota(iotaj[:], pattern=[[1, NDS]], base=0, channel_multiplier=0)

            # dummy activation: triggers the (single) ACT table load at t=0 so
            # it overlaps the input DMAs instead of sitting in the offsets chain
            warm = cst.tile([128, 1], F32, name="warm", tag="warm")
            nc.scalar.activation(warm[:], ones_col[:], AF.Relu)

            ones32r = cst.tile([1, 32], F32R, name="ones32r", tag="ones32r")
            nc.vector.tensor_copy(ones32r[:], ones32[:])
            ones_colr = cst.tile([128, 1], F32R, name="onesr", tag="onesr")
            nc.vector.tensor_copy(ones_colr[:], ones_col[:])
            iotaSf = cst.tile([128, NT], F32, name="iotaSf", tag="iotaSf")
            nc.vector.tensor_copy(iotaSf[:], iotaS[:])
            nc.vector.tensor_scalar(iotaSf[:], iotaSf[:], 0.5, None, ALU.add)
            iotaOf = cst.tile([128, NTT], F32, name="iotaOf", tag="iotaOf")
            nc.vector.tensor_copy(iotaOf[:], iotaO[:])
            nc.vector.tensor_scalar(iotaOf[:], iotaOf[:], -float(CQ), None, ALU.add)
            iotajf = rw.tile([1, NDS], F32, name="iotajf", tag="iotajf")
            nc.vector.tensor_copy(iotajf[:], iotaj[:])
            iotaj4 = rw.tile([1, NDS], F32, name="iotaj4", tag="iotaj4")
            nc.vector.tensor_scalar(iotaj4[:], iotajf[:], 4.0, None, ALU.add)
            wpjr = packedr[0:DPG, PK_WPJ - PK_QEND:PK_WPJ - PK_QEND + 1]

            # tent matrices (zeroed once; only static j-windows written later)
            S_all = pe_pool.tile([128, NT * NDS], F32R, name="S_all", tag="S_all")
            nc.gpsimd.memset(S_all[:].bitcast(F32), 0.0)
            OHT_all = pe_pool.tile([128, NTT * NDS], F32R, name="OHT_all", tag="OHT_all")
            nc.gpsimd.memset(OHT_all[:].bitcast(F32), 0.0)

            # persistent SBUF tiles that cross phase boundaries
            k_sb = pe_pool.tile([DPG, NDS], F32R, name="k_sb", tag="k_sb")
            qs_sb = pe_pool.tile([DPG, QS], F32R, name="qs_sb", tag="qs_sb")
            vT = [pe_pool.tile([128, DPG], F32R, name=f"vT{H}", tag=f"vT{H}") for H in range(2)]
            avn = pe_pool.tile([DPG, QS], F32R, name="avn", tag="avn")
            v_bc = pe_pool.tile([128, NDS], F32, name="v_bc", tag="v_bc")

            # ============ phase A: q, qs, offsets ============
            # phase-A PSUM pools are scoped to end at proj so their banks are
            # free for the attention pools while the kv path still runs
            psA_ctx = (
                tc.tile_pool(name="psA", bufs=2, space="PSUM"),
                tc.tile_pool(name="psA1", bufs=1, space="PSUM"),
                tc.tile_pool(name="psW", bufs=1, space="PSUM"),
            )
            psA = psA_ctx[0].__enter__()
            psA1 = psA_ctx[1].__enter__()
            psW = psA_ctx[2].__enter__()
            if True:
                def warm_pe(n, tag, rhs=None):
                    # rhs pins a data dependency so the tile scheduler cannot
                    # float the warmup ahead of the producer; low priority so
                    # ready real matmuls always win the PE slot
                    with tc.high_priority(offset=-100000):
                        for i in range(n):
                            r = wz[:, 0:256] if rhs is None else rhs
                            kdim = r.shape[0]
                            wp = psW.tile([1, r.shape[-1]], F32, name=f"wp_{tag}{i}", tag="wp")
                            nc.tensor.matmul(wp[:], ones_col[0:kdim, :], r)

                warm_pe(WARM_A, "a")

                # fused q+conv: acc[ch, m] directly from 3 stationary chunks
                # against stride-4 views of xg3 (kk pairs on partition halves)
                acc_ps = psA.tile([DPG, NDS], F32, name="acc_ps", tag="pA512")
                for c in range(3):
                    nc.tensor.matmul(acc_ps[:], packed0[:, PK_WC + DPG * c:PK_WC + DPG * (c + 1)],
                                     xg3[:, 2 * c:2 * c + 1021:DS],
                                     start=(c == 0), stop=(c == 2))
                # qs for this core's query half (scale folded in wqTs)
                pqs = psA.tile([DPG, QS], F32, name="pqs", tag="pA512")
                nc.tensor.matmul(pqs[:], packed0[0:DPG, PK_WQTS:PK_WQTS + DPG], xq[:])
                nc.scalar.copy(qs_sb[:], pqs[:])

                # x = acc + b (the only op reading the conv PSUM)
                acc = wk.tile([DPG, NDS], F32, name="conv_acc", tag="conv_acc")
                nc.vector.tensor_scalar(
                    acc[:], acc_ps[:], packed[0:DPG, PK_BDW:PK_BDW + 1], None, ALU.add)

                warm_pe(WARM_B, "b", xq[0:DPG, 0:256].bitcast(F32))
                # gelu returns the (xu, relu) pair; proj accumulates both so
                # the first matmul starts before the relu path finishes
                xu, glr = _erf_gelu(nc, wk, None, acc[:], [DPG, NDS])

                pproj = psA1.tile([1, NDS], F32, name="pproj", tag="small")
                nc.tensor.matmul(pproj[:], wpjr[:], xu[:], start=True, stop=False)
                nc.tensor.matmul(pproj[:], wpjr[:], glr[:], start=False, stop=True)
                warm_pe(WARM_C, "c", acc[:, 0:128])

                proj_sb = rw.tile([1, NDS], F32, name="proj_sb", tag="proj_sb")
                nc.vector.tensor_copy(proj_sb[:], pproj[:])
                for c in reversed(psA_ctx):
                    c.__exit__(None, None, None)
                # vg_raw = j + 4*tanh(proj) (scales folded into the tent builds)
                vg = rw.tile([1, NDS], F32, name="vg", tag="vg")
                _vg_from_proj(nc, rw, vg[:], proj_sb[:], iotaj4[:], [1, NDS])
                nc.gpsimd.partition_broadcast(v_bc[:], vg[:])

                # ---- attention-region PSUM pools (coexist with kv path) ----
                with (
                    tc.tile_pool(name="psB", bufs=1, space="PSUM") as psB,
                    tc.tile_pool(name="psE", bufs=1, space="PSUM") as psE,
                    tc.tile_pool(name="psE1", bufs=1, space="PSUM") as psE1,
                ):
                    # ---- bias tents first (they gate the phase-D matmuls) ----
                    with tc.high_priority(offset=1000):
                        for t in [3, 4, 5, 6, 7, 8, 0, 1, 2]:
                            jlo, jhi = _o_window(t)
                            if jhi <= jlo:
                                continue
                            w = jhi - jlo
                            wT = wk.tile([128, 48], F32, name="wT", tag="wT")
                            # w = c - kappa = (-A_O*vg) - (kappa - CQ)
                            nc.vector.tensor_scalar(
                                wT[:, 0:w], v_bc[:, jlo:jhi], -A_O, iotaOf[:, t:t + 1],
                                ALU.mult, ALU.subtract)
                            nc.vector.scalar_tensor_tensor(
                                wT[:, 0:w], wT[:, 0:w], -1.0, wT[:, 0:w], ALU.mult, ALU.min)
                            nc.scalar.activation(
                                OHT_all[:, NDS * t + jlo:NDS * t + jhi], wT[:, 0:w], AF.Relu, bias=1.0)

                    # ---- kv-gather tents ----
                    for t in range(NT):
                        jlo, jhi = _s_window(t)
                        w = jhi - jlo
                        wS = wk.tile([128, 48], F32, name="wS", tag="wS")
                        # w = ppix - pos = (A_S*vg) - (pos + 0.5)
                        nc.vector.tensor_scalar(
                            wS[:, 0:w], v_bc[:, jlo:jhi], A_S, iotaSf[:, t:t + 1],
                            ALU.mult, ALU.subtract)
                        nc.vector.scalar_tensor_tensor(
                            wS[:, 0:w], wS[:, 0:w], -1.0, wS[:, 0:w], ALU.mult, ALU.min)
                        nc.scalar.activation(
                            S_all[:, NDS * t + jlo:NDS * t + jhi], wS[:, 0:w], AF.Relu, bias=1.0)

                    # kv gather + k, v, vT
                    pkv = psB.tile([DPG, NDS], F32, name="pA256", tag="pA256")
                    for t in range(NT):
                        nc.tensor.matmul(pkv[:], xgT[:, DPG * t:DPG * (t + 1)],
                                         S_all[:, NDS * t:NDS * (t + 1)],
                                         start=(t == 0), stop=(t == NT - 1))
                    kv = wk.tile([DPG, NDS], F32R, name="kv", tag="kv")
                    nc.vector.tensor_copy(kv[:], pkv[:])

                    pk = psB.tile([DPG, NDS], F32, name="pA256", tag="pA256")
                    nc.tensor.matmul(pk[:], W(PK_WKT, DPG), kv[:])
                    nc.vector.tensor_copy(k_sb[:], pk[:])
                    pv = psB.tile([DPG, NDS], F32, name="pA256", tag="pA256")
                    nc.tensor.matmul(pv[:], W(PK_WVT, DPG), kv[:])
                    v_sb = wk.tile([DPG, NDS], F32, name="v_sb", tag="v_sb")
                    nc.vector.tensor_copy(v_sb[:], pv[:])

                    for H in range(2):
                        pt = psB.tile([128, DPG], F32, name="ptp", tag="pA256")
                        nc.tensor.transpose(pt[:], v_sb[:, H * 128:(H + 1) * 128], eye64)
                        nc.scalar.copy(vT[H][:], pt[:])

                    # ======== phase D: attention (bias accumulated in PSUM) ========
                    psims = {}
                    for h in range(2):
                        for H in range(2):
                            psims[(h, H)] = psE.tile([128, QS], F32, name=f"psim{h}{H}", tag=f"psim{h}{H}")
                    # per-group: bias matmuls (tent x Hankel window) for the
                    # kappa-tiles whose j-window intersects this H-half, then
                    # q.k closes the accumulation; softmax tail runs in i-halves
                    expT = {}
                    tlists = {H: [t for t in range(NTT)
                                  if _o_window(t)[0] < 128 * (H + 1) and _o_window(t)[1] > 128 * H]
                              for H in range(2)}
                    HF = QS // 2
                    for h in range(2):
                        for H in range(2):
                            for ti, t in enumerate(tlists[H]):
                                nc.tensor.matmul(
                                    psims[(h, H)][:],
                                    OHT_all[:, NDS * t + 128 * H:NDS * t + 128 * (H + 1)],
                                    hank[:, h * 1536 + 128 * t:h * 1536 + 128 * t + QS],
                                    start=(ti == 0), stop=False)
                            with tc.high_priority(offset=500):
                                nc.tensor.matmul(
                                    psims[(h, H)][:], k_sb[32 * h:32 * (h + 1), H * 128:(H + 1) * 128],
                                    qs_sb[32 * h:32 * (h + 1), :], start=False, stop=True)
                            for x in range(2):
                                et = wk.tile([128, HF], F32R, name=f"expT{h}{H}{x}", tag=f"expT{h}{H}{x}")
                                nc.scalar.activation(et[:], psims[(h, H)][:, HF * x:HF * (x + 1)], AF.Exp)
                                expT[(h, H, x)] = et

                        for x in range(2):
                            psum_s = psE1.tile([1, HF], F32, name=f"psum_s{h}{x}", tag="psum_s")
                            for H in range(2):
                                nc.tensor.matmul(psum_s[:], ones_colr[:], expT[(h, H, x)][:],
                                                 start=(H == 0), stop=(H == 1))
                            rs = rw.tile([1, HF], F32, name=f"rs{x}", tag=f"rs{x}")
                            nc.vector.reciprocal(rs[:], psum_s[:])
                            # broadcast 1/s to 32 partitions on Pool (SBUF only)
                            rsb = wk.tile([32, HF], F32, name=f"rsb{x}", tag=f"rsb{x}")
                            nc.gpsimd.partition_broadcast(rsb[:], rs[:])

                            pav = psE1.tile([32, HF], F32, name=f"pav{h}{x}", tag="pav")
                            for H in range(2):
                                nc.tensor.matmul(pav[:], vT[H][:, 32 * h:32 * (h + 1)],
                                                 expT[(h, H, x)][:],
                                                 start=(H == 0), stop=(H == 1))
                            nc.vector.tensor_tensor(avn[32 * h:32 * (h + 1), HF * x:HF * (x + 1)],
                                                    pav[:], rsb[:], ALU.mult)

                    ptags = ["psim00", "psim01", "psim10", "psim11"]
                    y_sbs = {m: wk.tile([128, QS], F32, name=f"y_sb{m}", tag=f"y_sb{m}")
                             for m in range(2)}
                    for x in range(2):
                        for m in range(2):
                            py = psE.tile([128, HF], F32, name=f"py{m}{x}", tag=ptags[2 * x + m])
                            nc.tensor.matmul(py[:], W(PK_WOT + m * 128, 128),
                                             avn[:, HF * x:HF * (x + 1)])
                            if m == 0:
                                nc.vector.tensor_copy(y_sbs[m][:, HF * x:HF * (x + 1)], py[:])
                            else:
                                nc.scalar.copy(y_sbs[m][:, HF * x:HF * (x + 1)], py[:])
                    nc.sync.dma_start(y_out.ap()[0:128, :], y_sbs[0][:])
                    nc.scalar.dma_start(y_out.ap()[128:256, 0:HF], y_sbs[1][:, 0:HF])
                    nc.scalar.dma_start(y_out.ap()[128:256, HF:QS], y_sbs[1][:, HF:QS])

    nc.compile()
    return nc


def _shard_inputs(inputs):
    """Build the 8 per-core input maps from the full inputs."""
    x = np.ascontiguousarray(inputs["x"][0])               # [256, 1024]
    wq, wk, wv = inputs["wq"], inputs["wk"], inputs["wv"]  # [4, 64, 64]
    wo = inputs["wo"]                                      # [256, 256]
    w_off_dw = inputs["w_off_dw"][:, 0, :]                 # [64, 6]
    b_off_dw = inputs["b_off_dw"]                          # [64]
    w_off_proj = inputs["w_off_proj"]                      # [64]
    w1 = inputs["cpb_w1"][:, 0].astype(np.float64)         # [64]
    b1 = inputs["cpb_b1"].astype(np.float64)
    w2 = inputs["cpb_w2"].astype(np.float64)
    b2 = inputs["cpb_b2"].astype(np.float64)
    w3 = inputs["cpb_w3"].astype(np.float64)               # [2, 64]
    b3 = inputs["cpb_b3"].astype(np.float64)

    f = np.float32

    # bias lookup tables + Hankel strips, one per query-half (weights-only)
    slabs = {}
    for qh in range(2):
        kk = np.arange(LTAB, dtype=np.float64)
        d = HQ * (kk - CQ + QS * qh)
        pos = np.sign(d) * np.log1p(np.abs(d))
        h1 = np.maximum(pos[:, None] * w1[None, :] + b1, 0.0)
        h2 = np.maximum(h1 @ w2.T + b2, 0.0)
        T = (h2 @ w3.T + b3).astype(f)                     # [LTAB, 2]
        sl = np.zeros((128, 2 * 1536), f)
        for o in range(2):
            sw = np.lib.stride_tricks.sliding_window_view(T[:, o], 1536)  # [129, 1536]
            sl[:, o * 1536:(o + 1) * 1536] = sw[0:128]
        slabs[qh] = sl

    base_packed = np.zeros((128, PCK), f)
    base_packed[0:DPG, PK_BDW] = b_off_dw
    base_packed[0:DPG, PK_EYE:PK_EYE + DPG] = np.eye(DPG, dtype=f)

    in_maps = []
    for c in range(NCORES):
        g, qh = c // 2, c % 2
        xg = np.ascontiguousarray(x[DPG * g:DPG * (g + 1)], dtype=f)
        # xg3[p, j] = x[p%64, j + p//64 - 1], zeros off the ends
        xg3 = np.zeros((128, N + 1), f)
        xg3[0:DPG, 1:N + 1] = xg
        xg3[DPG:128, 0:N] = xg
        xgT = np.zeros((128, NT * DPG), f)
        for t in range(NT):
            xgT[:, DPG * t:DPG * (t + 1)] = xg[:, 128 * t:128 * (t + 1)].T
        pk = base_packed.copy()
        p0 = np.zeros((128, P0K), f)
        p0[0:DPG, PK_WQTS:PK_WQTS + DPG] = wq[g].T * f(DH) ** f(-0.5)
        # Wc_c[(kk_half, i), ch] = wq[ch, i] * wdw[ch, 2c + kk_half]
        for c3 in range(3):
            for kh in range(2):
                p0[DPG * kh:DPG * (kh + 1), PK_WC + DPG * c3:PK_WC + DPG * (c3 + 1)] = (
                    wq[g].T * w_off_dw[None, :, 2 * c3 + kh])
        pr = np.zeros((128, PRK), f)
        pr[0:DPG, PK_WKT - PK_QEND:PK_WKT - PK_QEND + DPG] = wk[g].T
        pr[0:DPG, PK_WVT - PK_QEND:PK_WVT - PK_QEND + DPG] = wv[g].T
        pr[0:DPG, PK_WOT - PK_QEND:PK_WOT - PK_QEND + DIM] = wo[:, DPG * g:DPG * (g + 1)].T
        pr[0:DPG, PK_WPJ - PK_QEND] = 0.5 * w_off_proj
        m = {
            "packed0": p0,
            "packedr": pr,
            "xg3": xg3,
            "xq": np.ascontiguousarray(xg[:, QS * qh:QS * (qh + 1)]),
            "xgT": xgT,
            "packed": pk,
            "hank": slabs[qh],
        }
        in_maps.append(m)
    return in_maps


def kernel(**inputs):
    if "nc" not in _CACHED:
        _CACHED["nc"] = build_nc()
    nc = _CACHED["nc"]
    in_maps = _shard_inputs(inputs)
    res = bass_utils.run_bass_kernel_spmd(nc, in_maps, core_ids=list(range(NCORES)))
    ys = [res.results[c]["y"] for c in range(NCORES)]
    bo = inputs["bo"]
    out = np.zeros((1, DIM, N), np.float32)
    for qh in range(2):
        acc = np.zeros((DIM, QS), np.float64)
        for g in range(G):
            acc += ys[2 * g + qh]
        out[0, :, QS * qh:QS * (qh + 1)] = (acc + bo.astype(np.float64)[:, None]).astype(np.float32)
    return out


# revision 53
# speedup vs baseline: 1.0261x; 1.0261x over previous
"""DeformableAttention1D on 8 TRN2 NeuronCores via Bass/Tile.

Sharding: core c handles offset-group g=c//2 (64 of 256 channels, 2 of 8 heads)
and query-half qh=c%2 (512 of 1024 positions). Each core computes its group's
offsets/gather/bias/attention independently; the final output projection is
computed as a partial (wo sliced by group) and summed on the host (the
"all-reduce" of the output projection).

Key idea vs the straightforward implementation: the CPB relative-position-bias
MLP is a scalar->2 function F(d) of the signed distance d = gq_i - vgsp1_j,
and the query grid steps uniformly by h_q = 2/1023. Tabulating F on a grid
with spacing exactly h_q (a host-side weights-only precompute, like any other
weight repacking) turns the bias into

    bias[i, j] = (1-r_j) T[m_j + i] + r_j T[m_j + 1 + i],   m_j + r_j = c_j,

i.e. a per-column shifted window of the table. On device this is 9 matmuls of
"tent" interpolation one-hots max(0, 1-|c_j - kappa|) (stationary) against
host-precomputed Hankel slabs of T (moving), accumulated directly into the
attention-logit PSUM on top of q.k — the bias costs zero vector-engine work
in the attention phase. The tent matrices also implement the bilinear kv
grid-sample gather (zeros padding included). Tents are only materialized in
their static nonzero j-windows (~43 of 256 columns per kappa-tile).

Device numerics: fp32 data, fp32r matmuls (1 cycle/col vs 4 for fp32). The
ACT engine is restricted to ONE table set (natural_log_exp_and_others:
Exp/Ln/Relu/Copy/Identity/Square); tanh and erf(gelu) are composed from
Exp + DVE ops. Max bias interp error ~3e-4 in the logits.
"""
import os
import sys

sys.path.insert(0, "/opt/trn_rl_repo")

import numpy as np

import concourse.bacc as bacc
import concourse.bass as bass
import concourse.mybir as mybir
import concourse.tile as tile
import concourse.bass_utils as bass_utils

F32 = mybir.dt.float32
F32R = mybir.dt.float32r
I32 = mybir.dt.int32
U32 = mybir.dt.uint32
AF = mybir.ActivationFunctionType
ALU = mybir.AluOpType

# model dims (hardcoded per problem spec)
DIM = 256
N = 1024
G = 4
HEADS = 8
DH = 32
NDS = 256          # downsampled kv positions
QS = 512           # queries per core
DPG = 64           # channels per group
OFF_K = 6
DS = 4             # downsample stride
OFF_SCALE = 4.0
NCORES = 8

# bias lookup table
HQ = 2.0 / 1023.0  # query grid step == table spacing
CQ = 1040.0        # index offset so c_j = CQ - vgsp1_j/HQ stays in [0.9, 1057]
LTAB = 1664        # table length (slabs need up to 128*8+127+511 = 1662)
NTT = 9            # tent kappa-tiles (c_j+1 < 1152)
NT = 8             # x position tiles for the kv gather
A_S = 1024.0 / 255.0   # d ppix / d vg_raw  (|offset| < 4 -> +-4*A_S slack)
A_O = 1023.0 / 255.0   # -d c / d vg_raw

# A&S 7.1.25 3-term erf coefficients (|err| <= 2.5e-5)
ERF_P3 = 0.47047
ERF_A3 = [0.3480242, -0.0958798, 0.7478556]

# PE warmup chain lengths (tuned against the cost-model timeline)
WARM_A = 3
WARM_B = 6
WARM_C = 6

_CACHED = {}


def _s_window(t):
    """Static j-range where the kv-gather tent for position tile t can be
    nonzero: ppix_j in (128t-1, 128t+128), ppix = A_S*vg_raw-0.5, vg_raw in
    (j-4, j+4)."""
    jlo = max(0, int(np.floor((128 * t - 0.5) / A_S - 4)) - 1)
    jhi = min(NDS, int(np.ceil((128 * t + 128.5) / A_S + 4)) + 1)
    return jlo, jhi


def _o_window(t):
    """Static j-range where the bias tent for kappa tile t can be nonzero:
    c_j in (128t-1, 128t+128), c = CQ - A_O*vg_raw."""
    jlo = max(0, int(np.floor((912 - 128 * t) / A_O - 4)) - 1)
    jhi = min(NDS, int(np.ceil((1041 - 128 * t) / A_O + 4)) + 1)
    return jlo, jhi


def _patch_act_tables():
    """Restrict activation-table selection to the single set that covers all
    ACT functions used by this kernel, so exactly one table load is emitted
    (runtime table swaps do not work in this environment)."""
    import concourse.hw_specs as hw_specs

    if getattr(bacc, "_deform_act_patch", False):
        return
    orig = hw_specs.get_activation_tables

    keep = "natural_log_exp_and_others"

    def patched(module_arch):
        tabs = orig(module_arch)
        keep_funcs = tabs[keep]
        out = {}
        for name, funcs in tabs.items():
            if name == keep:
                out[name] = funcs
            else:
                out[name] = funcs - keep_funcs
        return out

    bacc.get_activation_tables = patched
    bacc._deform_act_patch = True


def _erf_gelu(nc, sb, out_ap, x_ap, shape):
    """Returns (xu, r) with xu + r = x * (1 + erf(x/sqrt(2))).

    A&S 7.1.25 3-term for erf(|x|/sqrt2) = 1 - poly(t)*exp(-x^2/2),
    t = 1/(1 + p*|x|/sqrt2). Sign handled without bit tricks:
    with u = poly*e in (0, 1]:  x*(1+erf(x)) = x*u + relu(2*x*(1-u)).
    """
    P, Nf = shape
    sq = sb.tile([P, Nf], F32, name="gelu_sq", tag="gelu_sq")
    nc.scalar.activation(sq[:], x_ap, AF.Square)
    e = sb.tile([P, Nf], F32, name="gelu_e", tag="gelu_e")
    nc.scalar.activation(e[:], sq[:], AF.Exp, scale=-0.5)
    ax = sb.tile([P, Nf], F32, name="gelu_ax", tag="gelu_ax")
    nc.vector.scalar_tensor_tensor(ax[:], x_ap, -1.0, x_ap, ALU.mult, ALU.max)
    t = sb.tile([P, Nf], F32, name="gelu_t", tag="gelu_t")
    nc.vector.tensor_scalar(t[:], ax[:], float(ERF_P3 / np.sqrt(2.0)), 1.0, ALU.mult, ALU.add)
    nc.vector.reciprocal(t[:], t[:])
    poly = sb.tile([P, Nf], F32, name="gelu_poly", tag="gelu_poly")
    # poly = ((a3 t + a2) t + a1) t
    nc.vector.tensor_scalar(poly[:], t[:], ERF_A3[2], ERF_A3[1], ALU.mult, ALU.add)
    nc.vector.tensor_tensor(poly[:], poly[:], t[:], ALU.mult)
    nc.vector.scalar_tensor_tensor(poly[:], poly[:], ERF_A3[0], t[:], ALU.add, ALU.mult)
    u = sb.tile([P, Nf], F32, name="gelu_u", tag="gelu_u")
    nc.vector.tensor_tensor(u[:], poly[:], e[:], ALU.mult)
    xu = sb.tile([P, Nf], F32R, name="gelu_xu", tag="gelu_xu")
    nc.vector.tensor_tensor(xu[:], x_ap, u[:], ALU.mult)
    w1 = sb.tile([P, Nf], F32, name="gelu_w1", tag="gelu_w1")
    nc.vector.tensor_tensor(w1[:], x_ap, xu[:].bitcast(F32), ALU.subtract)
    r = sb.tile([P, Nf], F32R, name="gelu_r", tag="gelu_r")
    nc.scalar.activation(r[:], w1[:], AF.Relu, scale=2.0)
    return xu, r


def _vg_from_proj(nc, sb, out_ap, x_ap, iotaj4_ap, shape):
    """out = j + 4*tanh(x), tanh = 1 - 2/(exp(2*clamp(x,-15,15))+1):
    out = (iotaj + 4) - 8/(exp(2*xc)+1)."""
    P, Nf = shape
    xc = sb.tile([P, Nf], F32, name="th_xc", tag="th_xc")
    nc.vector.tensor_scalar(xc[:], x_ap, -15.0, 15.0, ALU.max, ALU.min)
    e = sb.tile([P, Nf], F32, name="th_e", tag="th_e")
    nc.scalar.activation(e[:], xc[:], AF.Exp, scale=2.0)
    nc.vector.tensor_scalar(e[:], e[:], 1.0, None, ALU.add)
    r = sb.tile([P, Nf], F32, name="th_r", tag="th_r")
    nc.vector.reciprocal(r[:], e[:])
    nc.vector.scalar_tensor_tensor(out_ap, r[:], -8.0, iotaj4_ap, ALU.mult, ALU.add)


# packed0 (f32r, [128, P0K]): early stationaries. The offsets conv is folded
# into the q projection: acc[ch, m] = sum_{i,kk} wq[ch,i] wdw[ch,kk] x[i,4m+kk-1]
# = sum_c Wc_c^T Xc where Wc_c[(kk,i), ch] and Xc = stride-4 views of xg3.
PK_WQTS = 0         # wq[g].T * DH^-0.5     [64, 64]
PK_WC = 64          # conv stationaries     [128, 3*64]
P0K = 256
# packedr (f32r, [128, PRK]): remaining stationaries (offsets rel. PK_QEND)
PK_QEND = 320
PK_WKT = 320        # wk[g].T               [64, 64]
PK_WVT = 384        # wv[g].T               [64, 64]
PK_WOT = 448        # wo[:, group cols].T   [64, 256]
PK_WEND = 704
PK_WPJ = 704        # 0.5*w_off_proj        [64, 1]
PRK = PK_WEND - PK_QEND + 1
# packed (f32, [128, PCK]): non-matmul constants
PK_BDW = 0          # conv bias             [64, 1]
PK_EYE = 1          # identity              [64, 64]
PCK = 65


def build_nc():
    _patch_act_tables()
    nc = bacc.Bacc("TRN2", target_bir_lowering=False, debug=False, num_devices=NCORES)

    # ---- per-core DRAM inputs ----
    din = {}

    def dt_in(name, shape, dt=F32):
        din[name] = nc.dram_tensor(name, shape, dt, kind="ExternalInput")
        return din[name]

    dt_in("packed0", [128, P0K], F32R)
    dt_in("packedr", [128, PRK], F32R)
    dt_in("xg3", [128, N + 1], F32R)
    dt_in("xq", [DPG, QS], F32R)
    dt_in("xgT", [128, NT * DPG], F32R)
    dt_in("packed", [128, PCK])
    dt_in("hank", [128, 2 * 1536], F32R)
    y_out = nc.dram_tensor("y", [DIM, QS], F32, kind="ExternalOutput")

    with tile.TileContext(nc) as tc:
        with (
            tc.tile_pool(name="const", bufs=1) as cst,
            tc.tile_pool(name="work", bufs=2) as wk,
            tc.tile_pool(name="rows", bufs=1) as rw,
            tc.tile_pool(name="persist", bufs=1) as pe_pool,
        ):
            # ---- input DMAs; slab split across both HWDGE queues ----
            # xg3: x duplicated across partition halves with the kk-pair shift
            # baked in: xg3[p, j] = x[p%64, j + p//64 - 1] (zeros off the ends)
            xg3 = cst.tile([128, N + 1], F32R, name="xg3", tag="xg3")
            nc.sync.dma_start(xg3[:], din["xg3"].ap())
            packed0 = cst.tile([128, P0K], F32R, name="packed0", tag="packed0")
            nc.sync.dma_start(packed0[:], din["packed0"].ap())
            packed = cst.tile([128, PCK], F32, name="packed", tag="packed")
            nc.sync.dma_start(packed[:], din["packed"].ap())
            packedr = cst.tile([128, PRK], F32R, name="packedr", tag="packedr")
            nc.sync.dma_start(packedr[:], din["packedr"].ap())
            xq = cst.tile([DPG, QS], F32R, name="xq", tag="xq")
            nc.sync.dma_start(xq[:], din["xq"].ap())
            xgT = cst.tile([128, NT * DPG], F32R, name="xgT", tag="xgT")
            nc.sync.dma_start(xgT[:], din["xgT"].ap())
            # Hankel strip HH[k', c] = T[k' + c]; the 9 Hankel "slabs" per head
            # are its sliding 512-col windows, so one DMA covers them all
            hank = cst.tile([128, 2 * 1536], F32R, name="hank", tag="hank")
            nc.sync.dma_start(hank[:], din["hank"].ap())



            def W(col, width):
                return packedr[0:DPG, col - PK_QEND:col - PK_QEND + width]

            eye64 = packed[0:DPG, PK_EYE:PK_EYE + DPG]

            # small Pool ops + iotas first (the big tent memsets would
            # head-of-line-block the iotas and, through them, the DVE queue)
            ones_col = cst.tile([128, 1], F32, name="ones", tag="ones")
            nc.gpsimd.memset(ones_col[:], 1.0)
            ones32 = cst.tile([1, 32], F32, name="ones32", tag="ones32")
            nc.gpsimd.memset(ones32[:], 1.0)
            wz = cst.tile([128, 256], F32, name="wz", tag="wz")
            nc.gpsimd.memset(wz[:], 0.0)
            iotaS = cst.tile([128, NT], I32, name="iotaS", tag="iotaS")
            nc.gpsimd.iota(iotaS[:], pattern=[[128, NT]], base=0, channel_multiplier=1)
            iotaO = cst.tile([128, NTT], I32, name="iotaO", tag="iotaO")
            nc.gpsimd.iota(iotaO[:], pattern=[[128, NTT]], base=0, channel_multiplier=1)
            iotaj = rw.tile([1, NDS], I32, name="iotaj", tag="iotaj")
            nc.gpsimd.iota(iotaj[:], pattern=[[1, NDS]], base=0, channel_multiplier=0)

            # dummy activation: triggers the (single) ACT table load at t=0 so
            # it overlaps the input DMAs instead of sitting in the offsets chain
            warm = cst.tile([128, 1], F32, name="warm", tag="warm")
            nc.scalar.activation(warm[:], ones_col[:], AF.Relu)

            ones32r = cst.tile([1, 32], F32R, name="ones32r", tag="ones32r")
            nc.vector.tensor_copy(ones32r[:], ones32[:])
            ones_colr = cst.tile([128, 1], F32R, name="onesr", tag="onesr")
            nc.vector.tensor_copy(ones_colr[:], ones_col[:])
            iotaSf = cst.tile([128, NT], F32, name="iotaSf", tag="iotaSf")
            nc.vector.tensor_copy(iotaSf[:], iotaS[:])
            nc.vector.tensor_scalar(iotaSf[:], iotaSf[:], 0.5, None, ALU.add)
            iotaOf = cst.tile([128, NTT], F32, name="iotaOf", tag="iotaOf")
            nc.vector.tensor_copy(iotaOf[:], iotaO[:])
            nc.vector.tensor_scalar(iotaOf[:], iotaOf[:], -float(CQ), None, ALU.add)
            iotajf = rw.tile([1, NDS], F32, name="iotajf", tag="iotajf")
            nc.vector.tensor_copy(iotajf[:], iotaj[:])
            iotaj4 = rw.tile([1, NDS], F32, name="iotaj4", tag="iotaj4")
            nc.vector.tensor_scalar(iotaj4[:], iotajf[:], 4.0, None, ALU.add)
            wpjr = packedr[0:DPG, PK_WPJ - PK_QEND:PK_WPJ - PK_QEND + 1]

            # tent matrices (zeroed once; only static j-windows written later)
            S_all = pe_pool.tile([128, NT * NDS], F32R, name="S_all", tag="S_all")
            nc.gpsimd.memset(S_all[:].bitcast(F32), 0.0)
            OHT_all = pe_pool.tile([128, NTT * NDS], F32R, name="OHT_all", tag="OHT_all")
            nc.gpsimd.memset(OHT_all[:].bitcast(F32), 0.0)

            # persistent SBUF tiles that cross phase boundaries
            k_sb = pe_pool.tile([DPG, NDS], F32R, name="k_sb", tag="k_sb")
            qs_sb = pe_pool.tile([DPG, QS], F32R, name="qs_sb", tag="qs_sb")
            vT = [pe_pool.tile([128, DPG], F32R, name=f"vT{H}", tag=f"vT{H}") for H in range(2)]
            avn = pe_pool.tile([DPG, QS], F32R, name="avn", tag="avn")
            v_bc = pe_pool.tile([128, NDS], F32, name="v_bc", tag="v_bc")

            # ============ phase A: q, qs, offsets ============
            # phase-A PSUM pools are scoped to end at proj so their banks are
            # free for the attention pools while the kv path still runs
            psA_ctx = (
                tc.tile_pool(name="psA", bufs=2, space="PSUM"),
                tc.tile_pool(name="psA1", bufs=1, space="PSUM"),
                tc.tile_pool(name="psW", bufs=1, space="PSUM"),
            )
            psA = psA_ctx[0].__enter__()
            psA1 = psA_ctx[1].__enter__()
            psW = psA_ctx[2].__enter__()
            if True:
                def warm_pe(n, tag, rhs=None):
                    # rhs pins a data dependency so the tile scheduler cannot
                    # float the warmup ahead of the producer; low priority so
                    # ready real matmuls always win the PE slot
                    with tc.high_priority(offset=-100000):
                        for i in range(n):
                            r = wz[:, 0:256] if rhs is None else rhs
                            kdim = r.shape[0]
                            wp = psW.tile([1, r.shape[-1]], F32, name=f"wp_{tag}{i}", tag="wp")
                            nc.tensor.matmul(wp[:], ones_col[0:kdim, :], r)

                warm_pe(WARM_A, "a")

                # fused q+conv: acc[ch, m] directly from 3 stationary chunks
                # against stride-4 views of xg3 (kk pairs on partition halves)
                acc_ps = psA.tile([DPG, NDS], F32, name="acc_ps", tag="pA512")
                for c in range(3):
                    nc.tensor.matmul(acc_ps[:], packed0[:, PK_WC + DPG * c:PK_WC + DPG * (c + 1)],
                                     xg3[:, 2 * c:2 * c + 1021:DS],
                                     start=(c == 0), stop=(c == 2))
                # qs for this core's query half (scale folded in wqTs)
                pqs = psA.tile([DPG, QS], F32, name="pqs", tag="pA512")
                nc.tensor.matmul(pqs[:], packed0[0:DPG, PK_WQTS:PK_WQTS + DPG], xq[:])
                nc.scalar.copy(qs_sb[:], pqs[:])

                # x = acc + b (the only op reading the conv PSUM)
                acc = wk.tile([DPG, NDS], F32, name="conv_acc", tag="conv_acc")
                nc.vector.tensor_scalar(
                    acc[:], acc_ps[:], packed[0:DPG, PK_BDW:PK_BDW + 1], None, ALU.add)

                warm_pe(WARM_B, "b", xq[0:DPG, 0:256].bitcast(F32))
                # gelu returns the (xu, relu) pair; proj accumulates both so
                # the first matmul starts before the relu path finishes
                xu, glr = _erf_gelu(nc, wk, None, acc[:], [DPG, NDS])

                pproj = psA1.tile([1, NDS], F32, name="pproj", tag="small")
                nc.tensor.matmul(pproj[:], wpjr[:], xu[:], start=True, stop=False)
                nc.tensor.matmul(pproj[:], wpjr[:], glr[:], start=False, stop=True)
                warm_pe(WARM_C, "c", acc[:, 0:128])

                proj_sb = rw.tile([1, NDS], F32, name="proj_sb", tag="proj_sb")
                nc.vector.tensor_copy(proj_sb[:], pproj[:])
                for c in reversed(psA_ctx):
                    c.__exit__(None, None, None)
                # vg_raw = j + 4*tanh(proj) (scales folded into the tent builds)
                vg = rw.tile([1, NDS], F32, name="vg", tag="vg")
                _vg_from_proj(nc, rw, vg[:], proj_sb[:], iotaj4[:], [1, NDS])
                nc.gpsimd.partition_broadcast(v_bc[:], vg[:])

                # ---- attention-region PSUM pools (coexist with kv path) ----
                with (
                    tc.tile_pool(name="psB", bufs=1, space="PSUM") as psB,
                    tc.tile_pool(name="psE", bufs=1, space="PSUM") as psE,
                    tc.tile_pool(name="psE1", bufs=1, space="PSUM") as psE1,
                ):
                    # ---- bias tents first (they gate the phase-D matmuls) ----
                    with tc.high_priority(offset=1000):
                        for t in [4, 0, 5, 1, 6, 2, 7, 3, 8]:
                            jlo, jhi = _o_window(t)
                            if jhi <= jlo:
                                continue
                            w = jhi - jlo
                            wT = wk.tile([128, 48], F32, name="wT", tag="wT")
                            # w = c - kappa = (-A_O*vg) - (kappa - CQ)
                            nc.vector.tensor_scalar(
                                wT[:, 0:w], v_bc[:, jlo:jhi], -A_O, iotaOf[:, t:t + 1],
                                ALU.mult, ALU.subtract)
                            nc.vector.scalar_tensor_tensor(
                                wT[:, 0:w], wT[:, 0:w], -1.0, wT[:, 0:w], ALU.mult, ALU.min)
                            nc.scalar.activation(
                                OHT_all[:, NDS * t + jlo:NDS * t + jhi], wT[:, 0:w], AF.Relu, bias=1.0)

                    # ---- kv-gather tents ----
                    for t in range(NT):
                        jlo, jhi = _s_window(t)
                        w = jhi - jlo
                        wS = wk.tile([128, 48], F32, name="wS", tag="wS")
                        # w = ppix - pos = (A_S*vg) - (pos + 0.5)
                        nc.vector.tensor_scalar(
                            wS[:, 0:w], v_bc[:, jlo:jhi], A_S, iotaSf[:, t:t + 1],
                            ALU.mult, ALU.subtract)
                        nc.vector.scalar_tensor_tensor(
                            wS[:, 0:w], wS[:, 0:w], -1.0, wS[:, 0:w], ALU.mult, ALU.min)
                        nc.scalar.activation(
                            S_all[:, NDS * t + jlo:NDS * t + jhi], wS[:, 0:w], AF.Relu, bias=1.0)

                    # kv gather + k, v, vT
                    pkv = psB.tile([DPG, NDS], F32, name="pA256", tag="pA256")
                    for t in range(NT):
                        nc.tensor.matmul(pkv[:], xgT[:, DPG * t:DPG * (t + 1)],
                                         S_all[:, NDS * t:NDS * (t + 1)],
                                         start=(t == 0), stop=(t == NT - 1))
                    kv = wk.tile([DPG, NDS], F32R, name="kv", tag="kv")
                    nc.vector.tensor_copy(kv[:], pkv[:])

                    pk = psB.tile([DPG, NDS], F32, name="pA256", tag="pA256")
                    nc.tensor.matmul(pk[:], W(PK_WKT, DPG), kv[:])
                    nc.vector.tensor_copy(k_sb[:], pk[:])
                    pv = psB.tile([DPG, NDS], F32, name="pA256", tag="pA256")
                    nc.tensor.matmul(pv[:], W(PK_WVT, DPG), kv[:])
                    v_sb = wk.tile([DPG, NDS], F32, name="v_sb", tag="v_sb")
                    nc.vector.tensor_copy(v_sb[:], pv[:])

                    for H in range(2):
                        pt = psB.tile([128, DPG], F32, name="ptp", tag="pA256")
                        nc.tensor.transpose(pt[:], v_sb[:, H * 128:(H + 1) * 128], eye64)
                        nc.scalar.copy(vT[H][:], pt[:])

                    # ======== phase D: attention (bias accumulated in PSUM) ========
                    psims = {}
                    for h in range(2):
                        for H in range(2):
                            psims[(h, H)] = psE.tile([128, QS], F32, name=f"psim{h}{H}", tag=f"psim{h}{H}")
                    # per-group: bias matmuls (tent x Hankel window) for the
                    # kappa-tiles whose j-window intersects this H-half, then
                    # q.k closes the accumulation; softmax tail runs in i-halves
                    expT = {}
                    tlists = {H: [t for t in range(NTT)
                                  if _o_window(t)[0] < 128 * (H + 1) and _o_window(t)[1] > 128 * H]
                              for H in range(2)}
                    HF = QS // 2
                    for h in range(2):
                        for H in range(2):
                            for ti, t in enumerate(tlists[H]):
                                nc.tensor.matmul(
                                    psims[(h, H)][:],
                                    OHT_all[:, NDS * t + 128 * H:NDS * t + 128 * (H + 1)],
                                    hank[:, h * 1536 + 128 * t:h * 1536 + 128 * t + QS],
                                    start=(ti == 0), stop=False)
                            with tc.high_priority(offset=500):
                                nc.tensor.matmul(
                                    psims[(h, H)][:], k_sb[32 * h:32 * (h + 1), H * 128:(H + 1) * 128],
                                    qs_sb[32 * h:32 * (h + 1), :], start=False, stop=True)
                            for x in range(2):
                                et = wk.tile([128, HF], F32R, name=f"expT{h}{H}{x}", tag=f"expT{h}{H}{x}")
                                nc.scalar.activation(et[:], psims[(h, H)][:, HF * x:HF * (x + 1)], AF.Exp)
                                expT[(h, H, x)] = et

                        for x in range(2):
                            psum_s = psE1.tile([1, HF], F32, name=f"psum_s{h}{x}", tag="psum_s")
                            for H in range(2):
                                nc.tensor.matmul(psum_s[:], ones_colr[:], expT[(h, H, x)][:],
                                                 start=(H == 0), stop=(H == 1))
                            rs = rw.tile([1, HF], F32R, name=f"rs{x}", tag=f"rs{x}")
                            with nc.allow_low_precision(reason="f32r bits == f32; PE rounds anyway"):
                                nc.vector.reciprocal(rs[:], psum_s[:])
                            pav = psE1.tile([32, HF], F32, name=f"pav{h}{x}", tag="pav")
                            for H in range(2):
                                nc.tensor.matmul(pav[:], vT[H][:, 32 * h:32 * (h + 1)],
                                                 expT[(h, H, x)][:],
                                                 start=(H == 0), stop=(H == 1))
                            rsb = psE1.tile([32, HF], F32, name=f"rsb{h}{x}", tag="rsb")
                            nc.tensor.matmul(rsb[:], ones32r[:], rs[:])
                            pav_sb = wk.tile([32, HF], F32, name=f"pav_sb{x}", tag=f"pav_sb{x}")
                            nc.scalar.copy(pav_sb[:], pav[:])
                            nc.vector.tensor_tensor(
                                avn[32 * h:32 * (h + 1), HF * x:HF * (x + 1)],
                                pav_sb[:], rsb[:], ALU.mult)

                    ptags = ["psim00", "psim01", "psim10", "psim11"]
                    y_sbs = {m: wk.tile([128, QS], F32, name=f"y_sb{m}", tag=f"y_sb{m}")
                             for m in range(2)}
                    for x in range(2):
                        for m in range(2):
                            py = psE.tile([128, HF], F32, name=f"py{m}{x}", tag=ptags[2 * x + m])
                            nc.tensor.matmul(py[:], W(PK_WOT + m * 128, 128),
                                             avn[:, HF * x:HF * (x + 1)])
                            if m == 0:
                                nc.vector.tensor_copy(y_sbs[m][:, HF * x:HF * (x + 1)], py[:])
                            else:
                                nc.scalar.copy(y_sbs[m][:, HF * x:HF * (x + 1)], py[:])
                    nc.sync.dma_start(y_out.ap()[0:128, :], y_sbs[0][:])
                    nc.scalar.dma_start(y_out.ap()[128:256, 0:HF], y_sbs[1][:, 0:HF])
                    nc.scalar.dma_start(y_out.ap()[128:256, HF:QS], y_sbs[1][:, HF:QS])

    nc.compile()
    return nc


def _shard_inputs(inputs):
    """Build the 8 per-core input maps from the full inputs."""
    x = np.ascontiguousarray(inputs["x"][0])               # [256, 1024]
    wq, wk, wv = inputs["wq"], inputs["wk"], inputs["wv"]  # [4, 64, 64]
    wo = inputs["wo"]                                      # [256, 256]
    w_off_dw = inputs["w_off_dw"][:, 0, :]                 # [64, 6]
    b_off_dw = inputs["b_off_dw"]                          # [64]
    w_off_proj = inputs["w_off_proj"]                      # [64]
    w1 = inputs["cpb_w1"][:, 0].astype(np.float64)         # [64]
    b1 = inputs["cpb_b1"].astype(np.float64)
    w2 = inputs["cpb_w2"].astype(np.float64)
    b2 = inputs["cpb_b2"].astype(np.float64)
    w3 = inputs["cpb_w3"].astype(np.float64)               # [2, 64]
    b3 = inputs["cpb_b3"].astype(np.float64)

    f = np.float32

    # bias lookup tables + Hankel strips, one per query-half (weights-only)
    slabs = {}
    for qh in range(2):
        kk = np.arange(LTAB, dtype=np.float64)
        d = HQ * (kk - CQ + QS * qh)
        pos = np.sign(d) * np.log1p(np.abs(d))
        h1 = np.maximum(pos[:, None] * w1[None, :] + b1, 0.0)
        h2 = np.maximum(h1 @ w2.T + b2, 0.0)
        T = (h2 @ w3.T + b3).astype(f)                     # [LTAB, 2]
        sl = np.zeros((128, 2 * 1536), f)
        for o in range(2):
            sw = np.lib.stride_tricks.sliding_window_view(T[:, o], 1536)  # [129, 1536]
            sl[:, o * 1536:(o + 1) * 1536] = sw[0:128]
        slabs[qh] = sl

    base_packed = np.zeros((128, PCK), f)
    base_packed[0:DPG, PK_BDW] = b_off_dw
    base_packed[0:DPG, PK_EYE:PK_EYE + DPG] = np.eye(DPG, dtype=f)

    in_maps = []
    for c in range(NCORES):
        g, qh = c // 2, c % 2
        xg = np.ascontiguousarray(x[DPG * g:DPG * (g + 1)], dtype=f)
        # xg3[p, j] = x[p%64, j + p//64 - 1], zeros off the ends
        xg3 = np.zeros((128, N + 1), f)
        xg3[0:DPG, 1:N + 1] = xg
        xg3[DPG:128, 0:N] = xg
        xgT = np.zeros((128, NT * DPG), f)
        for t in range(NT):
            xgT[:, DPG * t:DPG * (t + 1)] = xg[:, 128 * t:128 * (t + 1)].T
        pk = base_packed.copy()
        p0 = np.zeros((128, P0K), f)
        p0[0:DPG, PK_WQTS:PK_WQTS + DPG] = wq[g].T * f(DH) ** f(-0.5)
        # Wc_c[(kk_half, i), ch] = wq[ch, i] * wdw[ch, 2c + kk_half]
        for c3 in range(3):
            for kh in range(2):
                p0[DPG * kh:DPG * (kh + 1), PK_WC + DPG * c3:PK_WC + DPG * (c3 + 1)] = (
                    wq[g].T * w_off_dw[None, :, 2 * c3 + kh])
        pr = np.zeros((128, PRK), f)
        pr[0:DPG, PK_WKT - PK_QEND:PK_WKT - PK_QEND + DPG] = wk[g].T
        pr[0:DPG, PK_WVT - PK_QEND:PK_WVT - PK_QEND + DPG] = wv[g].T
        pr[0:DPG, PK_WOT - PK_QEND:PK_WOT - PK_QEND + DIM] = wo[:, DPG * g:DPG * (g + 1)].T
        pr[0:DPG, PK_WPJ - PK_QEND] = 0.5 * w_off_proj
        m = {
            "packed0": p0,
            "packedr": pr,
            "xg3": xg3,
            "xq": np.ascontiguousarray(xg[:, QS * qh:QS * (qh + 1)]),
            "xgT": xgT,
            "packed": pk,
            "hank": slabs[qh],
        }
        in_maps.append(m)
    return in_maps


def kernel(**inputs):
    if "nc" not in _CACHED:
        _CACHED["nc"] = build_nc()
    nc = _CACHED["nc"]
    in_maps = _shard_inputs(inputs)
    res = bass_utils.run_bass_kernel_spmd(nc, in_maps, core_ids=list(range(NCORES)))
    ys = [res.results[c]["y"] for c in range(NCORES)]
    bo = inputs["bo"]
    out = np.zeros((1, DIM, N), np.float32)
    for qh in range(2):
        acc = np.zeros((DIM, QS), np.float64)
        for g in range(G):
            acc += ys[2 * g + qh]
        out[0, :, QS * qh:QS * (qh + 1)] = (acc + bo.astype(np.float64)[:, None]).astype(np.float32)
    return out
